# revision 1
# baseline (speedup 1.0000x reference)
"""Trainium2 Bass kernel for nn_Decoder_15934328668408.

Attention-decoder scan: per step t (255 steps), a 1-unit dense + LSTM cell +
temporal attention over T=256 encoder states, hidden sizes M=P=256, batch 256.

Strategy
--------
- Data-parallel over batch across 8 NeuronCores (32 batch rows per core),
  parameters replicated, zero collectives. Host gathers per-core outputs.
- Host precomputes everything step-invariant (exactly what the reference
  hoists, plus linear-algebra refactors):
    * Ue = encoder_h @ Wu + bu + bw, transposed to [m-partitions, (b,t)]
    * ehWd1[b,t] = encoder_h[b,t,:] @ Wd[1:,0]  (lets the per-step Dense(1)
      y = [x, ctx] @ Wd + bd become a dot of beta with ehWd1 -- so the
      context vector itself is never needed inside the scan)
    * final ctx / output head on host from the last step's beta.
- Device per step (per core, 32 batch slots):
    y (DVE dot) -> z = [h;y,1] @ [Wr;Wk;bl] (PE) -> transpose -> gates
    (ACT tanh; sigmoid via 0.5*tanh(x/2)+0.5) -> h,s update (DVE) ->
    dscT = Ww^T [h;s] (PE) -> arg = Ue + dsc (DVE tensor_scalar, per-
    partition bias) -> tanh (ACT, the bottleneck: 2.1M elems/step) ->
    l = Wv^T arg (PE, 4 column-group-tiled masked matmuls) -> exp/sum/recip.

Batch slot s lives at PSUM row r(s) = 32*(s//8) + s%8 (column-group q=s//8).
"""

import numpy as np

B, T, M, P = 256, 256, 256, 256
NCORES = 8
BL = B // NCORES          # 32 batch rows per core
NSTEPS = T - 1            # 255
F32 = None                # set lazily (mybir import)

_CACHE = {}


def _r_of_s(s):
    return 32 * (s // 8) + (s % 8)


def _prep_core_inputs(data, encoder_h, Wd, Wu, bu, bw, c):
    """Per-core input arrays (host-side precompute)."""
    b0 = c * BL
    eh = np.ascontiguousarray(encoder_h[b0:b0 + BL]).astype(np.float32)  # (32,T,M)
    # Ue = eh @ Wu + bu + bw, laid out [128, 2, BL*T] with
    # Ue_t[p, mt, s*T + t] = Ue[s, t, mt*128 + p]
    Ue = eh.reshape(BL * T, M) @ Wu + (bu + bw)[None, :]                 # (BL*T, M)
    Ue_t = np.ascontiguousarray(
        Ue.reshape(BL, T, 2, 128).transpose(3, 2, 0, 1).reshape(128, 2, BL * T)
    ).astype(np.float32)
    # ehWd1[r(s), t] = eh[s,t,:] @ Wd[1:,0]  on scrambled rows
    ehwd1 = np.zeros((128, T), np.float32)
    dot = eh.reshape(BL * T, M) @ Wd[1:, 0]                              # (BL*T,)
    dot = dot.reshape(BL, T)
    # xscr[r(s), t] = Wd0 * data[s, t]  (x-contribution to y at step t)
    xscr = np.zeros((128, T), np.float32)
    for s in range(BL):
        r = _r_of_s(s)
        ehwd1[r] = dot[s]
        xscr[r, :T - 1] = Wd[0, 0] * data[b0 + s, :, 0]
    return {"ue": Ue_t, "ehwd1": ehwd1, "xscr": xscr}


def _prep_shared(Wd, bd, Wk, Wr, bl, Ww, Wv):
    """Weight tensors shared by all cores, pre-laid-out for the device."""
    wr = np.ascontiguousarray(
        Wr.reshape(2, 128, 4 * P).transpose(1, 0, 2)).astype(np.float32)  # [128,2,1024]
    # z += y*Wk + 1*bl_eff ; bl_eff folds bd through Wk (y_true = y_dev + bd)
    bl_eff = bl + bd[0] * Wk[0]
    wkbl = np.stack([Wk[0], bl_eff]).astype(np.float32)                   # [2,1024]
    ww = np.ascontiguousarray(
        Ww.reshape(4, 128, M).transpose(1, 0, 2)).astype(np.float32)      # [128,4,256]
    # masked Wv for column-group-tiled l-matmuls:
    # wv_sl[p, kt, j, c] = Wv[kt*128+p] if c == j else 0
    wv_sl = np.zeros((128, 2, 8, 8), np.float32)
    for kt in range(2):
        for j in range(8):
            wv_sl[:, kt, j, j] = Wv[kt * 128:(kt + 1) * 128, 0]
    ident = np.eye(128, dtype=np.float32)
    return {"wr": wr, "wkbl": wkbl, "ww": ww, "wv_sl": wv_sl, "ident": ident}


def build_module(nsteps=NSTEPS, cut=0):
    """Build and compile the per-core Bass module. Returns (nc, names).

    cut (debug): 0 = full; N>0 emits only the first N numbered stages per step.
    """
    import concourse.bass as bass
    import concourse.bacc as bacc
    import concourse.tile as tile
    import concourse.mybir as mybir
    from contextlib import ExitStack

    F32 = mybir.dt.float32
    AF = mybir.ActivationFunctionType
    OP = mybir.AluOpType

    nc = bacc.Bacc("TRN2", target_bir_lowering=False, debug=False)

    din = {}
    for name, shape in [
        ("ue", (128, 2, BL * T)), ("ehwd1", (128, T)), ("xscr", (128, T)),
        ("wr", (128, 2, 4 * P)), ("wkbl", (2, 4 * P)), ("ww", (128, 4, M)),
        ("wv_sl", (128, 2, 8, 8)), ("ident", (128, 128)),
    ]:
        din[name] = nc.dram_tensor(name, shape, F32, kind="ExternalInput").ap()
    d_h = nc.dram_tensor("h_out", (128, 2, BL), F32, kind="ExternalOutput").ap()
    d_beta = nc.dram_tensor("beta_out", (128, T), F32, kind="ExternalOutput").ap()

    with tile.TileContext(nc) as tc, ExitStack() as stk:
        const = stk.enter_context(tc.tile_pool(name="const", bufs=1))
        state = stk.enter_context(tc.tile_pool(name="state", bufs=2))
        scr = stk.enter_context(tc.tile_pool(name="scr", bufs=2))
        psum = stk.enter_context(tc.tile_pool(name="psum", bufs=1, space="PSUM"))
        psum2 = stk.enter_context(tc.tile_pool(name="psum2", bufs=1, space="PSUM"))

        # ---- resident inputs ----
        ue = const.tile([128, 2, BL * T], F32)
        ehwd1 = const.tile([128, T], F32)
        xscr = const.tile([128, T], F32)
        wr = const.tile([128, 2, 4 * P], F32)
        wkbl = const.tile([2, 4 * P], F32)
        ww = const.tile([128, 4, M], F32)
        wv_sl = const.tile([128, 2, 8, 8], F32)
        ident = const.tile([128, 128], F32)
        arg = const.tile([128, 2, BL * T], F32)   # tanh workspace
        for t_, n_ in [(ue, "ue"), (ehwd1, "ehwd1"), (xscr, "xscr"), (wr, "wr"),
                       (wkbl, "wkbl"), (ww, "ww"), (wv_sl, "wv_sl"), (ident, "ident")]:
            nc.sync.dma_start(out=t_[:], in_=din[n_][:])

        # persistent PSUM for l (memset once; masked matmuls rewrite rows)
        l_ps = psum.tile([128, T], F32, tag="l")
        nc.vector.memset(l_ps[:], 0.0)

        # yOnes: row 0 = y^T (per step), row 1 = const 1.0
        y_ones = const.tile([2, BL], F32)
        nc.vector.memset(y_ones[:], 1.0)  # row 0 overwritten per step; row 1 stays 1.0

        # initial state
        hT = state.tile([128, 2, BL], F32, tag="hT")
        sT = state.tile([128, 2, BL], F32, tag="sT")
        nc.vector.memset(hT[:], 0.0)
        nc.vector.memset(sT[:], 0.0)

        expl = None
        recip = None

        for t in range(nsteps):
            # ---- 1. y (scalar per slot) --------------------------------
            y_col = scr.tile([128, 1], F32, tag="y_col")
            if t == 0 or (cut and cut < 6):
                nc.vector.tensor_copy(y_col[:], xscr[:, 0:1])
            else:
                ydot = scr.tile([128, T], F32, tag="ydot")
                nc.vector.tensor_mul(ydot[:], expl[:], ehwd1[:])
                w = T
                while w > 1:
                    w //= 2
                    nc.vector.tensor_add(ydot[:, 0:w], ydot[:, 0:w], ydot[:, w:2 * w])
                nc.vector.tensor_scalar_mul(y_col[:], ydot[:, 0:1], recip[:])
                nc.vector.tensor_add(y_col[:], y_col[:], xscr[:, t:t + 1])
            yT_ps = psum2.tile([1, 128], F32, tag="yT")
            nc.tensor.transpose(yT_ps[:], y_col[:], ident[:])
            nc.vector.tensor_copy(
                y_ones[0:1, :],
                yT_ps.rearrange("p (q j) -> p q j", q=4)[:, :, 0:8])

            if cut and cut < 2:
                continue
            # ---- 2. z = [h; y,1] @ [Wr; Wk,bl]  -> (32, 1024) ----------
            z_ps = psum2.tile([BL, 4 * P], F32, tag="z")
            for nh in range(2):
                sl = slice(nh * 512, (nh + 1) * 512)
                nc.tensor.matmul(z_ps[:, sl], hT[:, 0, :], wr[:, 0, sl],
                                 start=True, stop=False)
                nc.tensor.matmul(z_ps[:, sl], hT[:, 1, :], wr[:, 1, sl],
                                 start=False, stop=False)
                nc.tensor.matmul(z_ps[:, sl], y_ones[:], wkbl[:, sl],
                                 start=False, stop=True)
            z_sb = scr.tile([BL, 4 * P], F32, tag="z_sb")
            nc.vector.tensor_copy(z_sb[:], z_ps[:])
            zT_ps = psum2.tile([128, 8, BL], F32, tag="zT")
            for j in range(8):
                nc.tensor.transpose(zT_ps[:, j, :],
                                    z_sb[:, j * 128:(j + 1) * 128],
                                    ident[0:BL, 0:BL])

            if cut and cut < 3:
                continue
            # ---- 3. gates (sigmoid via tanh), state update -------------
            ti = scr.tile([128, 2, BL], F32, tag="ti")
            tf = scr.tile([128, 2, BL], F32, tag="tf")
            tg = scr.tile([128, 2, BL], F32, tag="tg")
            to = scr.tile([128, 2, BL], F32, tag="to")
            nc.scalar.activation(ti[:], zT_ps[:, 0:2, :], AF.Tanh, scale=0.5)
            nc.scalar.activation(tf[:], zT_ps[:, 2:4, :], AF.Tanh, scale=0.5)
            nc.scalar.activation(tg[:], zT_ps[:, 4:6, :], AF.Tanh, scale=1.0)
            nc.scalar.activation(to[:], zT_ps[:, 6:8, :], AF.Tanh, scale=0.5)
            # s' = 0.5*((tf*s + s) + (ti*tg + tg)); h' = 0.5*(to*th + th)
            u = scr.tile([128, 2, BL], F32, tag="u")
            v = scr.tile([128, 2, BL], F32, tag="v")
            nc.vector.tensor_mul(u[:], tf[:], sT[:])
            nc.vector.tensor_add(u[:], u[:], sT[:])
            nc.vector.tensor_mul(v[:], ti[:], tg[:])
            nc.vector.tensor_add(v[:], v[:], tg[:])
            sT = state.tile([128, 2, BL], F32, tag="sT")
            nc.vector.tensor_add(sT[:], u[:], v[:])
            nc.vector.tensor_scalar_mul(sT[:], sT[:], 0.5)
            tanh_s = scr.tile([128, 2, BL], F32, tag="tanh_s")
            nc.scalar.activation(tanh_s[:], sT[:], AF.Tanh)
            hT = state.tile([128, 2, BL], F32, tag="hT")
            nc.vector.tensor_mul(hT[:], to[:], tanh_s[:])
            nc.vector.tensor_add(hT[:], hT[:], tanh_s[:])
            nc.vector.tensor_scalar_mul(hT[:], hT[:], 0.5)

            if cut and cut < 4:
                continue
            # ---- 4. dscT = Ww^T [h; s]  -> [128, 2, 32] ----------------
            dscT_ps = psum2.tile([128, 2, BL], F32, tag="dscT")
            for mc in range(2):
                sl = slice(mc * 128, (mc + 1) * 128)
                for kt in range(4):
                    rhs = hT[:, kt, :] if kt < 2 else sT[:, kt - 2, :]
                    nc.tensor.matmul(dscT_ps[:, mc, :], ww[:, kt, sl], rhs,
                                     start=(kt == 0), stop=(kt == 3))
            dscT = scr.tile([128, 2, BL], F32, tag="dscT_sb")
            nc.vector.tensor_copy(dscT[:], dscT_ps[:])

            if cut and cut < 5:
                continue
            # ---- 5. attention: arg = tanh(Ue + dsc); l = Wv^T arg ------
            for g in range(4):
                for j in range(8):
                    s = g * 8 + j
                    sl = slice(s * T, (s + 1) * T)
                    for mt in range(2):
                        nc.vector.tensor_scalar_add(
                            arg[:, mt, sl], ue[:, mt, sl], dscT[:, mt, s:s + 1])
                if cut == 41:
                    continue
                gsl = slice(g * 8 * T, (g + 1) * 8 * T)
                for mt in range(2):
                    nc.scalar.activation(arg[:, mt, gsl], arg[:, mt, gsl], AF.Tanh)
                if cut == 42:
                    continue
                for j in range(8):
                    s = g * 8 + j
                    sl = slice(s * T, (s + 1) * T)
                    rows = slice(32 * g, 32 * g + 8)
                    nc.tensor.matmul(
                        l_ps[rows, :], wv_sl[:, 0, j, :], arg[:, 0, sl],
                        start=(j == 0), stop=False,
                        tile_position=(0, 32 * g), skip_group_check=True)
                    nc.tensor.matmul(
                        l_ps[rows, :], wv_sl[:, 1, j, :], arg[:, 1, sl],
                        start=False, stop=(j == 7),
                        tile_position=(0, 32 * g), skip_group_check=True)

            if cut and cut < 6:
                continue
            # ---- 6. softmax pieces ------------------------------------
            expl = scr.tile([128, T], F32, tag="expl")
            nc.scalar.activation(expl[:], l_ps[:], AF.Exp)
            stree = scr.tile([128, T], F32, tag="stree")
            nc.vector.tensor_add(stree[:, 0:T // 2], expl[:, 0:T // 2],
                                 expl[:, T // 2:T])
            w = T // 2
            while w > 1:
                w //= 2
                nc.vector.tensor_add(stree[:, 0:w], stree[:, 0:w], stree[:, w:2 * w])
            # Newton: x <- x*(2 - s*x), x0 = 1/256  (s in ~[150, 420])
            recip = scr.tile([128, 1], F32, tag="recip")
            ntmp = scr.tile([128, 1], F32, tag="ntmp")
            nc.vector.memset(recip[:], 1.0 / 256.0)
            for _ in range(4):
                nc.vector.tensor_mul(ntmp[:], stree[:, 0:1], recip[:])
                nc.vector.tensor_scalar_mul(ntmp[:], ntmp[:], -1.0)
                nc.vector.tensor_scalar_add(ntmp[:], ntmp[:], 2.0)
                nc.vector.tensor_mul(recip[:], recip[:], ntmp[:])

        # ---- outputs ----
        beta = const.tile([128, T], F32)
        if cut:
            nc.vector.memset(beta[:], 0.0)
        else:
            nc.vector.tensor_scalar_mul(beta[:], expl[:], recip[:])
        nc.sync.dma_start(out=d_beta[:], in_=beta[:])
        nc.sync.dma_start(out=d_h[:], in_=hT[:])

    nc.compile()
    return nc


def _run_on_device(nc, in_maps, trace=False):
    from concourse.bass_utils import run_bass_kernel_spmd
    return run_bass_kernel_spmd(
        nc, in_maps, core_ids=list(range(len(in_maps))), trace=trace)


def _full_kernel(inputs, nsteps=NSTEPS, trace=False):
    data = np.asarray(inputs["data"], np.float32)
    encoder_h = np.asarray(inputs["encoder_h"], np.float32)
    Wd = np.asarray(inputs["Wd"], np.float32)
    bd = np.asarray(inputs["bd"], np.float32)
    Wk = np.asarray(inputs["Wk"], np.float32)
    Wr = np.asarray(inputs["Wr"], np.float32)
    bl = np.asarray(inputs["bl"], np.float32)
    Ww = np.asarray(inputs["Ww"], np.float32)
    bw = np.asarray(inputs["bw"], np.float32)
    Wu = np.asarray(inputs["Wu"], np.float32)
    bu = np.asarray(inputs["bu"], np.float32)
    Wv = np.asarray(inputs["Wv"], np.float32)

    key = nsteps
    if key not in _CACHE:
        _CACHE[key] = build_module(nsteps)
    nc = _CACHE[key]

    shared = _prep_shared(Wd, bd, Wk, Wr, bl, Ww, Wv)
    in_maps = []
    for c in range(NCORES):
        m = _prep_core_inputs(data, encoder_h, Wd, Wu, bu, bw, c)
        m.update(shared)
        in_maps.append(m)

    res = _run_on_device(nc, in_maps, trace=trace)

    # ---- host-side gather + epilogue ----
    Wvb = np.asarray(inputs["Wvb"], np.float32)
    bvb = np.asarray(inputs["bvb"], np.float32)
    Wwb = np.asarray(inputs["Wwb"], np.float32)
    bwb = np.asarray(inputs["bwb"], np.float32)

    out = np.zeros((B, 1, P), np.float32)
    rows = np.array([_r_of_s(s) for s in range(BL)])
    for c in range(NCORES):
        r = res.results[c]
        h = r["h_out"].transpose(2, 1, 0).reshape(BL, P)       # (32, 256)
        beta = r["beta_out"][rows]                             # (32, T)
        eh = encoder_h[c * BL:(c + 1) * BL]                    # (32, T, M)
        ctx = np.einsum("st,stm->sm", beta.astype(np.float32), eh)
        cat = np.concatenate([h, ctx], axis=-1)                # (32, 512)
        head = (cat @ Wvb + bvb) @ Wwb + bwb                   # (32, 256)
        out[c * BL:(c + 1) * BL, 0, :] = head
    return out, res


def kernel(**inputs):
    out, _ = _full_kernel(inputs, nsteps=NSTEPS, trace=False)
    return out



# revision 9
# speedup vs baseline: 1.4105x; 1.4105x over previous
"""Trainium2 Bass kernel for nn_Decoder_15934328668408 (v2).

Attention-decoder scan: per step t (255 steps), a 1-unit dense + LSTM cell +
temporal attention over T=256 encoder states, hidden sizes M=P=256, batch 256.

Strategy
--------
- Data-parallel over batch across 8 NeuronCores (32 batch rows per core),
  parameters replicated, zero collectives. Host gathers per-core outputs.
- Host precomputes step-invariant tensors:
    * Ue = encoder_h @ Wu + bu + bw in bf16, laid out t-major
      [m-part, mt, t, s] so the per-step dsc broadcast-add is a handful of
      large stride-0-broadcast DVE ops at 4x rate.
    * ehWd1[b,t] = encoder_h[b,t,:] @ Wd[1:,0] so the per-step Dense(1)
      y = [x, ctx] @ Wd + bd becomes a dot of softmax numerator with ehWd1.
    * LSTM weights transposed (z computed directly in [gate, slot] layout,
      no transposes in the loop), with the sigmoid-via-tanh 0.5 scales and
      the doubled-state compensation folded in host-side.
- Doubled state: device carries S=2s (f32) and H=2h (bf16);
  sig(x) = (tanh(x/2)+1)/2 so all four gates use one plain tanh
  activation, and state updates are three fused scalar_tensor_tensor ops.
- Per step: y (fused mult-add) -> yT (PE transpose) -> z y-part (PE, fp32)
  -> gates (one ACT tanh over [128,8,32] PSUM) -> S/H update (fused DVE)
  -> dsc (PE bf16) -> per t-chunk: arg = Ue + dsc (DVE stt broadcast, bf16)
  -> tanh (ACT, in place bf16) -> l (PE masked bf16 matmuls, strided rhs)
  -> exp (ACT, accum_out -> denominator) -> numerator dot (DVE stt accum).
  The recurrent-part matmuls of z for step t+1 run on the PE during step
  t's attention (h_t is final before attention starts).

Batch slot s of a core lives at row r(s) = 32*(s//8) + s%8 for the softmax
tensors (l/expl/ehwd1/xscr); s-indexed free dims are in natural order.
"""

import numpy as np
import ml_dtypes

B, T, M, P = 256, 256, 256, 256
NCORES = 8
BL = B // NCORES          # 32 batch rows per core
NSTEPS = T - 1            # 255
CHUNKS = ((0, 64), (64, 128), (128, 192), (192, 256))

_CACHE = {}


def _r_of_s(s):
    return 32 * (s // 8) + (s % 8)


def _gate_scale():
    """Per-column scale for z: 0.5 for i,f,o gates (sigmoid-via-tanh), 1 for g."""
    gs = np.ones(4 * P, np.float32)
    gs[0 * P:2 * P] = 0.5     # i, f
    gs[3 * P:4 * P] = 0.5     # o
    return gs


def _prep_core_inputs(data, encoder_h, Wd, Wu, bu, bw, c):
    b0 = c * BL
    eh = np.ascontiguousarray(encoder_h[b0:b0 + BL]).astype(np.float32)  # (32,T,M)
    Ue = eh.reshape(BL * T, M) @ Wu + (bu + bw)[None, :]                 # (BL*T, M)
    # ue2[p, t, mt*32+s] = Ue[s, t, mt*128 + p]   (t-major, bf16)
    ue2 = np.ascontiguousarray(
        Ue.reshape(BL, T, 2, 128).transpose(3, 1, 2, 0).reshape(128, T, 2 * BL)
    ).astype(ml_dtypes.bfloat16)                                          # (128,T,64)
    ehwd1 = np.zeros((128, T), np.float32)
    dot = (eh.reshape(BL * T, M) @ Wd[1:, 0]).reshape(BL, T)
    xscr = np.zeros((128, T), np.float32)
    for s in range(BL):
        r = _r_of_s(s)
        ehwd1[r] = dot[s]
        xscr[r, :T - 1] = Wd[0, 0] * data[b0 + s, :, 0]
    return {"ue2": ue2, "ehwd1": ehwd1, "xscr": xscr}


def _prep_shared(Wd, bd, Wk, Wr, bl, Ww, Wv):
    gs = _gate_scale()
    # H = 2h compensation on recurrent weights, plus gate scale.
    wr_eff = Wr * 0.5 * gs[None, :]                                       # (P, 4P)
    wrT2 = np.ascontiguousarray(
        wr_eff.reshape(2, 128, 8, 128).transpose(1, 0, 2, 3)
    ).astype(ml_dtypes.bfloat16)                                          # (128,2,8,128)
    wk_eff = Wk[0] * gs
    bl_eff = (bl + bd[0] * Wk[0]) * gs
    wkblT = np.ascontiguousarray(
        np.stack([wk_eff, bl_eff]).reshape(2, 8, 128)).astype(np.float32)  # (2,8,128)
    ww_eff = Ww * 0.5                                                     # H=2h, S=2s
    wwT = np.ascontiguousarray(
        ww_eff.reshape(4, 128, 2, 128).transpose(1, 0, 2, 3)
    ).astype(ml_dtypes.bfloat16)                                          # (128,4,2,128)
    wv_sl = np.zeros((128, 2, 8, 8), np.float32)
    for kt in range(2):
        for j in range(8):
            wv_sl[:, kt, j, j] = Wv[kt * 128:(kt + 1) * 128, 0]
    wv_sl = wv_sl.astype(ml_dtypes.bfloat16)
    ident = np.eye(128, dtype=np.float32)
    return {"wrT2": wrT2, "wkblT": wkblT, "wwT": wwT, "wv_sl": wv_sl,
            "ident": ident}


def build_module(nsteps=NSTEPS):
    import concourse.bass as bass
    import concourse.bacc as bacc
    import concourse.tile as tile
    import concourse.mybir as mybir
    from contextlib import ExitStack

    F32 = mybir.dt.float32
    BF16 = mybir.dt.bfloat16
    AF = mybir.ActivationFunctionType
    OP = mybir.AluOpType

    nc = bacc.Bacc("TRN2", target_bir_lowering=False, debug=False)

    din = {}
    for name, shape, dt in [
        ("ue2", (128, T, 2 * BL), BF16), ("ehwd1", (128, T), F32),
        ("xscr", (128, T), F32), ("wrT2", (128, 2, 8, 128), BF16),
        ("wkblT", (2, 8, 128), F32), ("wwT", (128, 4, 2, 128), BF16),
        ("wv_sl", (128, 2, 8, 8), BF16), ("ident", (128, 128), F32),
    ]:
        din[name] = nc.dram_tensor(name, shape, dt, kind="ExternalInput").ap()
    d_h = nc.dram_tensor("h_out", (128, 2, BL), F32, kind="ExternalOutput").ap()
    d_beta = nc.dram_tensor("beta_out", (128, T), F32, kind="ExternalOutput").ap()

    with tile.TileContext(nc) as tc, ExitStack() as stk:
        const = stk.enter_context(tc.tile_pool(name="const", bufs=1))
        state = stk.enter_context(tc.tile_pool(name="state", bufs=2))
        scr = stk.enter_context(tc.tile_pool(name="scr", bufs=2))
        pA = stk.enter_context(tc.tile_pool(name="pA", bufs=1, space="PSUM"))
        pB = stk.enter_context(tc.tile_pool(name="pB", bufs=1, space="PSUM"))
        pC = stk.enter_context(tc.tile_pool(name="pC", bufs=1, space="PSUM"))
        pD = stk.enter_context(tc.tile_pool(name="pD", bufs=1, space="PSUM"))

        # ---- resident inputs ----
        ue2 = const.tile([128, T, 2 * BL], BF16)
        ehwd1 = const.tile([128, T], F32)
        xscr = const.tile([128, T], F32)
        wrT2 = const.tile([128, 2, 8, 128], BF16)
        wkblT = const.tile([2, 8, 128], F32)
        wwT = const.tile([128, 4, 2, 128], BF16)
        wv_sl = const.tile([128, 2, 8, 8], BF16)
        ident = const.tile([128, 128], F32)
        for t_, n_ in [(ue2, "ue2"), (ehwd1, "ehwd1"), (xscr, "xscr"),
                       (wrT2, "wrT2"), (wkblT, "wkblT"), (wwT, "wwT"),
                       (wv_sl, "wv_sl"), (ident, "ident")]:
            nc.sync.dma_start(out=t_[:], in_=din[n_][:])

        arg = const.tile([128, T, 2 * BL], BF16)   # tanh workspace
        expl = const.tile([128, T], F32)
        prod = const.tile([128, T], F32)           # stt throwaway out
        beta = const.tile([128, T], F32)

        l_ps = pA.tile([128, T], F32, tag="l")
        nc.vector.memset(l_ps[:], 0.0)
        zT_ps = pB.tile([128, 8, BL], F32, tag="zT")
        dscT_ps = pC.tile([128, 2, BL], F32, tag="dscT")
        yT_ps = pD.tile([1, 128], F32, tag="yT")

        y_ones = const.tile([2, BL], F32)
        nc.vector.memset(y_ones[:], 1.0)   # row 0 overwritten per step

        S = state.tile([128, 2, BL], F32, tag="S")
        nc.vector.memset(S[:], 0.0)
        H = None

        num_tot = None
        rden = None

        for t in range(nsteps):
            # ---- y (per-slot scalar), y^T ---------------------------------
            y_col = scr.tile([128, 1], F32, tag="y_col")
            if t == 0:
                nc.vector.tensor_copy(y_col[:], xscr[:, 0:1])
            else:
                nc.vector.scalar_tensor_tensor(
                    out=y_col[:], in0=num_tot[:], scalar=rden[:], op0=OP.mult,
                    in1=xscr[:, t:t + 1], op1=OP.add)
            nc.tensor.transpose(yT_ps[:], y_col[:], ident[:])
            nc.vector.tensor_copy(
                y_ones[0:1, :],
                yT_ps.rearrange("p (q j) -> p q j", q=4)[:, :, 0:8])

            # ---- z y-part: zT += Wk^T y + bl  (recurrent part accumulated
            #      during the previous step's attention) --------------------
            for gt in range(8):
                nc.tensor.matmul(zT_ps[:, gt, :], wkblT[:, gt, :], y_ones[:],
                                 start=(t == 0), stop=True,
                                 skip_group_check=True)

            # ---- gates: one tanh over all of zT ---------------------------
            gates = scr.tile([128, 8, BL], BF16, tag="gates")
            nc.scalar.activation(gates[:], zT_ps[:], AF.Tanh)
            ti, tf = gates[:, 0:2, :], gates[:, 2:4, :]
            tg, to = gates[:, 4:6, :], gates[:, 6:8, :]

            # ---- state update (S=2s, H=2h) --------------------------------
            u = scr.tile([128, 2, BL], F32, tag="u")
            v = scr.tile([128, 2, BL], F32, tag="v")
            nc.vector.scalar_tensor_tensor(
                out=u[:], in0=tf, scalar=1.0, op0=OP.add, in1=S[:], op1=OP.mult)
            nc.vector.scalar_tensor_tensor(
                out=v[:], in0=ti, scalar=1.0, op0=OP.add, in1=tg, op1=OP.mult)
            S = state.tile([128, 2, BL], F32, tag="S")
            nc.vector.scalar_tensor_tensor(
                out=S[:], in0=u[:], scalar=0.5, op0=OP.mult, in1=v[:], op1=OP.add)
            th = scr.tile([128, 2, BL], BF16, tag="th")
            nc.scalar.activation(th[:], S[:], AF.Tanh, scale=0.5)
            H = state.tile([128, 2, BL], BF16, tag="H")
            nc.vector.scalar_tensor_tensor(
                out=H[:], in0=to, scalar=1.0, op0=OP.add, in1=th[:], op1=OP.mult)
            Sb = scr.tile([128, 2, BL], BF16, tag="Sb")
            nc.gpsimd.tensor_copy(Sb[:], S[:])

            # ---- dsc = Ww^T [H; S] ---------------------------------------
            for mc in range(2):
                for kt in range(4):
                    rhs = H[:, kt, :] if kt < 2 else Sb[:, kt - 2, :]
                    nc.tensor.matmul(dscT_ps[:, mc, :], wwT[:, kt, mc, :], rhs,
                                     start=(kt == 0), stop=(kt == 3),
                                     skip_group_check=True)
            dscb = scr.tile([128, 1, 2 * BL], BF16, tag="dscb")
            nc.vector.tensor_copy(dscb[:, 0, :], dscT_ps[:])

            # ---- attention, chunked over t --------------------------------
            den4 = scr.tile([128, 4], F32, tag="den4")
            num4 = scr.tile([128, 4], F32, tag="num4")
            for ci, (t0, t1) in enumerate(CHUNKS):
                w = t1 - t0
                dsc_b = dscb[:, 0:1, :].broadcast_to((128, w, 2 * BL))
                nc.vector.scalar_tensor_tensor(
                    out=arg[:, t0:t1, :], in0=ue2[:, t0:t1, :],
                    scalar=0.0, op0=OP.bypass, in1=dsc_b, op1=OP.add)
                nc.scalar.activation(arg[:, t0:t1, :], arg[:, t0:t1, :],
                                     AF.Tanh)
                for g in range(4):
                    for kt in range(2):
                        for j in range(8):
                            s = g * 8 + j
                            nc.tensor.matmul(
                                l_ps[32 * g:32 * g + 8, t0:t1],
                                wv_sl[:, kt, j, :],
                                arg[:, t0:t1, kt * BL + s],
                                start=(kt == 0 and j == 0),
                                stop=(kt == 1 and j == 7),
                                tile_position=(0, 32 * g),
                                skip_group_check=True)
                # recurrent z-part for step t+1 while ACT works on tanh
                if ci == 0 and t + 1 < nsteps:
                    for gt in range(8):
                        for ht in range(2):
                            nc.tensor.matmul(
                                zT_ps[:, gt, :], wrT2[:, ht, gt, :], H[:, ht, :],
                                start=(ht == 0), stop=False,
                                skip_group_check=True)
                # exp of the previous chunk (keeps ACT from stalling on PE)
                if ci > 0:
                    p0, p1 = CHUNKS[ci - 1]
                    nc.scalar.activation(expl[:, p0:p1], l_ps[:, p0:p1], AF.Exp,
                                         accum_out=den4[:, ci - 1:ci])
                    nc.vector.scalar_tensor_tensor(
                        out=prod[:, p0:p1], in0=expl[:, p0:p1], scalar=0.0,
                        op0=OP.bypass, in1=ehwd1[:, p0:p1], op1=OP.mult,
                        accum_out=num4[:, ci - 1:ci])
            p0, p1 = CHUNKS[-1]
            nc.scalar.activation(expl[:, p0:p1], l_ps[:, p0:p1], AF.Exp,
                                 accum_out=den4[:, 3:4])
            nc.vector.scalar_tensor_tensor(
                out=prod[:, p0:p1], in0=expl[:, p0:p1], scalar=0.0,
                op0=OP.bypass, in1=ehwd1[:, p0:p1], op1=OP.mult,
                accum_out=num4[:, 3:4])

            # ---- softmax numerator/denominator totals ---------------------
            den_tot = scr.tile([128, 1], F32, tag="den_tot")
            num_tot = scr.tile([128, 1], F32, tag="num_tot")
            nc.vector.tensor_reduce(den_tot[:], den4[:], mybir.AxisListType.X,
                                    OP.add)
            nc.vector.tensor_reduce(num_tot[:], num4[:], mybir.AxisListType.X,
                                    OP.add)
            rden = scr.tile([128, 1], F32, tag="rden")
            nc.vector.reciprocal_approx_fast(out=rden[:], in_=den_tot[:])

        # ---- outputs ----
        nc.vector.tensor_scalar_mul(beta[:], expl[:], rden[:])
        h_f32 = const.tile([128, 2, BL], F32)
        nc.vector.tensor_copy(h_f32[:], H[:])
        nc.sync.dma_start(out=d_beta[:], in_=beta[:])
        nc.sync.dma_start(out=d_h[:], in_=h_f32[:])

    nc.compile()
    return nc


def _run_on_device(nc, in_maps, trace=False):
    from concourse.bass_utils import run_bass_kernel_spmd
    return run_bass_kernel_spmd(
        nc, in_maps, core_ids=list(range(len(in_maps))), trace=trace)


def _full_kernel(inputs, nsteps=NSTEPS, trace=False):
    data = np.asarray(inputs["data"], np.float32)
    encoder_h = np.asarray(inputs["encoder_h"], np.float32)
    Wd = np.asarray(inputs["Wd"], np.float32)
    bd = np.asarray(inputs["bd"], np.float32)
    Wk = np.asarray(inputs["Wk"], np.float32)
    Wr = np.asarray(inputs["Wr"], np.float32)
    bl = np.asarray(inputs["bl"], np.float32)
    Ww = np.asarray(inputs["Ww"], np.float32)
    bw = np.asarray(inputs["bw"], np.float32)
    Wu = np.asarray(inputs["Wu"], np.float32)
    bu = np.asarray(inputs["bu"], np.float32)
    Wv = np.asarray(inputs["Wv"], np.float32)

    key = nsteps
    if key not in _CACHE:
        _CACHE[key] = build_module(nsteps)
    nc = _CACHE[key]

    shared = _prep_shared(Wd, bd, Wk, Wr, bl, Ww, Wv)
    in_maps = []
    for c in range(NCORES):
        m = _prep_core_inputs(data, encoder_h, Wd, Wu, bu, bw, c)
        m.update(shared)
        in_maps.append(m)

    res = _run_on_device(nc, in_maps, trace=trace)

    # ---- host-side gather + epilogue ----
    Wvb = np.asarray(inputs["Wvb"], np.float32)
    bvb = np.asarray(inputs["bvb"], np.float32)
    Wwb = np.asarray(inputs["Wwb"], np.float32)
    bwb = np.asarray(inputs["bwb"], np.float32)

    out = np.zeros((B, 1, P), np.float32)
    rows = np.array([_r_of_s(s) for s in range(BL)])
    for c in range(NCORES):
        r = res.results[c]
        # h_out holds H = 2h in [128, 2, 32] transposed layout
        h = 0.5 * r["h_out"].transpose(2, 1, 0).reshape(BL, P)  # (32, 256)
        beta = r["beta_out"][rows]                              # (32, T)
        eh = encoder_h[c * BL:(c + 1) * BL]                     # (32, T, M)
        ctx = np.einsum("st,stm->sm", beta.astype(np.float32), eh)
        cat = np.concatenate([h, ctx], axis=-1)                 # (32, 512)
        head = (cat @ Wvb + bvb) @ Wwb + bwb                    # (32, 256)
        out[c * BL:(c + 1) * BL, 0, :] = head
    return out, res


def kernel(**inputs):
    out, _ = _full_kernel(inputs, nsteps=NSTEPS, trace=False)
    return out
